# revision 9
# baseline (speedup 1.0000x reference)
"""Trainium2 Bass kernel for nn_Attention (Bahdanau-style attention decode step).

Reference computation (per batch b):
    h_proj  = hidden[b] @ W_h                      # [1, H]
    e_proj  = enc[b] @ W_e                         # [S, H]
    energy  = tanh(e_proj + h_proj + bias)         # [S, H]
    attn    = energy @ v                           # [S]
    w       = softmax(attn)                        # [S]
    context = w @ enc[b]                           # [E]

Sharding: data-parallel over batch on 8 cores (4 batches/core), no collectives.

Per-core kernel strategy (all matmuls in float32r = full-speed ~tf32):
  - enc tiles DMA'd in natural [s=128p, e] layout (contiguous rows).
  - PE transpose (128x128 blocks) produces encT [e=128p, s] for the main matmul.
  - Main matmul emits transposed energy [h=128p, s=512] per h-tile; the
    (h_proj + bias) term is a per-partition scalar there, so it folds into the
    tanh via ACT's bias operand.
  - attention = v.T @ energyT directly on PE (v as 128x1 stationary).
  - exp on ACT with accum_out accumulating the softmax denominator for free.
    Max-subtraction is skipped: |logits| <= sum|v| ~ 16, safe in fp32.
  - exp row transposed back to [s=128p, 1] columns on PE; context accumulates
    on PE against the natural-layout enc tiles already in SBUF.
"""

import numpy as np

HIDDEN = 1024
ENC = 1024
BATCH = 32
SEQ = 2048
NCORES = 8
B_LOC = BATCH // NCORES  # 4

S_CHUNK = 512
N_CHUNK = SEQ // S_CHUNK  # 4
SUBS = S_CHUNK // 128  # 4
ET = ENC // 128  # 8 e-tiles
HT = HIDDEN // 128  # 8 h-tiles

_CACHED_NC = None


def build_bass(b_loc=B_LOC, seq=SEQ, repeat=1):
    import concourse.mybir as mybir
    import concourse.tile as tile
    from concourse import bacc
    from concourse.bass import ts
    from concourse.masks import make_identity

    n_chunk = seq // S_CHUNK

    nc = bacc.Bacc()
    R = mybir.dt.float32r
    F = mybir.dt.float32
    AF = mybir.ActivationFunctionType

    hidden = nc.dram_tensor("hidden", [b_loc, HIDDEN], R, kind="ExternalInput")
    enc = nc.dram_tensor("enc", [b_loc, seq, ENC], R, kind="ExternalInput")
    attn_w = nc.dram_tensor("attn_w", [HIDDEN + ENC, HIDDEN], R, kind="ExternalInput")
    attn_b = nc.dram_tensor("attn_b", [HIDDEN], F, kind="ExternalInput")
    v_w = nc.dram_tensor("v_w", [HIDDEN], R, kind="ExternalInput")
    out = nc.dram_tensor("out", [b_loc, ENC], F, kind="ExternalOutput")

    import concourse.bass as bass

    with tile.TileContext(nc) as tc:
        with (
            tc.tile_pool(name="weights", bufs=1) as w_pool,
            tc.tile_pool(name="consts", bufs=1) as const_pool,
            tc.tile_pool(name="nat", bufs=8) as nat_pool,
            tc.tile_pool(name="encT", bufs=16) as encT_pool,
            tc.tile_pool(name="energyT", bufs=6) as energyT_pool,
            tc.tile_pool(name="small", bufs=8) as small_pool,
            tc.tile_pool(name="ps_tp", bufs=2, space="PSUM") as tp_pool,
            tc.tile_pool(name="ps_main", bufs=2, space="PSUM") as main_pool,
            tc.tile_pool(name="ps_attn", bufs=2, space="PSUM") as attn_pool,
            tc.tile_pool(name="ps_ctx", bufs=1, space="PSUM") as ctx_pool,
        ):
            # ---- constants / weights ----
            ident_f = const_pool.tile([128, 128], F, tag="ident_f")
            make_identity(nc, ident_f[:])
            ident = const_pool.tile([128, 128], R)
            nc.vector.tensor_copy(ident[:], ident_f[:])

            w_h = w_pool.tile([128, ET, HIDDEN], R, tag="w_h")
            w_e = w_pool.tile([128, ET, HIDDEN], R, tag="w_e")
            # attn_w rows [0:1024] are W_h, [1024:2048] are W_e
            nc.sync.dma_start(
                w_h[:], attn_w[0:HIDDEN, :].rearrange("(j p) h -> p j h", p=128)
            )
            nc.sync.dma_start(
                w_e[:],
                attn_w[HIDDEN : HIDDEN + ENC, :].rearrange("(j p) h -> p j h", p=128),
            )

            # v as [h=128p, j] columns
            vT2 = const_pool.tile([128, HT, 2], R, tag="vT2")
            for k in range(2):
                nc.gpsimd.dma_start(
                    out=vT2[:, :, k], in_=v_w[:].rearrange("(j p) -> p j", p=128)
                )

            # ---- preamble: hb[b, h] = hidden[b] @ W_h + attn_b, laid out as
            # hbT [h=128p, j, b] per-partition bias columns ----
            hidden_nat = const_pool.tile([b_loc, HIDDEN], R, tag="hidden_nat")
            nc.sync.dma_start(hidden_nat[:], hidden[:, :])

            hiddenT = const_pool.tile([128, ET, b_loc], R, tag="hiddenT")
            for j in range(ET):
                pt = tp_pool.tile([128, 512], R, tag="tp")
                nc.tensor.matmul(
                    pt[:, 0:b_loc],
                    hidden_nat[0:b_loc, ts(j, 128)],
                    ident[0:b_loc, 0:b_loc],
                    is_transpose=True,
                    start=True,
                    stop=True,
                )
                nc.vector.tensor_copy(hiddenT[:, j, :], pt[:, 0:b_loc])

            hb_ps = ctx_pool.tile([b_loc, HIDDEN], F, tag="ctx")
            for n in range(2):
                for j in range(ET):
                    nc.tensor.matmul(
                        hb_ps[:, ts(n, 512)],
                        hiddenT[:, j, :],
                        w_h[:, j, ts(n, 512)],
                        start=(j == 0),
                        stop=(j == ET - 1),
                    )
            attnb_sb = const_pool.tile([b_loc, HIDDEN], F, tag="attnb")
            nc.gpsimd.dma_start(
                out=attnb_sb[:], in_=attn_b[:].partition_broadcast(b_loc)
            )
            hb_nat = const_pool.tile([b_loc, HIDDEN], R, tag="hb_nat")
            nc.vector.tensor_add(hb_nat[:], hb_ps[:], attnb_sb[:])

            hbT = const_pool.tile([128, HT, b_loc], R, tag="hbT")
            for j in range(HT):
                pt = tp_pool.tile([128, 512], R, tag="tp")
                nc.tensor.matmul(
                    pt[:, 0:b_loc],
                    hb_nat[0:b_loc, ts(j, 128)],
                    ident[0:b_loc, 0:b_loc],
                    is_transpose=True,
                    start=True,
                    stop=True,
                )
                nc.vector.tensor_copy(hbT[:, j, :], pt[:, 0:b_loc])

            # ---- main loop ----
            for b in [bb for _ in range(repeat) for bb in range(b_loc)]:
                ctx_ps = ctx_pool.tile([1, ENC], F, tag="ctx")
                zparts = small_pool.tile([2, n_chunk], F, tag="zparts")
                for c in range(n_chunk):
                    # load natural enc tiles [s=128, e=1024]
                    nats = []
                    for t in range(SUBS):
                        nat = nat_pool.tile([128, ENC], R, tag="nat")
                        s0 = c * S_CHUNK + t * 128
                        nc.sync.dma_start(nat[:], enc[b, s0 : s0 + 128, :])
                        nats.append(nat)
                    # transpose to encT [e=128p, s=512] per e-tile
                    encTs = []
                    for j in range(ET):
                        pt = tp_pool.tile([128, 512], R, tag="tp")
                        for t in range(SUBS):
                            nc.tensor.matmul(
                                pt[:, ts(t, 128)],
                                nats[t][:, ts(j, 128)],
                                ident[:],
                                is_transpose=True,
                                start=(t == 0),
                                stop=(t == SUBS - 1),
                            )
                        eT = encT_pool.tile([128, S_CHUNK], R, tag="encT")
                        nc.vector.tensor_copy(eT[:], pt[:])
                        encTs.append(eT)
                    # main matmul + tanh + attention accumulation
                    attn_ps = attn_pool.tile([2, S_CHUNK], F, tag="attn")
                    for i in range(HT):
                        pm = main_pool.tile([128, S_CHUNK], F, tag="main")
                        for j in range(ET):
                            nc.tensor.matmul(
                                pm[:],
                                w_e[:, j, ts(i, 128)],
                                encTs[j][:],
                                start=(j == 0),
                                stop=(j == ET - 1),
                            )
                        et = energyT_pool.tile([128, S_CHUNK], R, tag="energyT")
                        nc.scalar.activation(
                            et[:], pm[:], AF.Tanh, bias=hbT[:, i, b : b + 1]
                        )
                        nc.tensor.matmul(
                            attn_ps[:],
                            vT2[:, i, :],
                            et[:],
                            start=(i == 0),
                            stop=(i == HT - 1),
                        )
                    # softmax exp (no max-sub; |logit| <= 16) + denominator
                    exp_row = small_pool.tile([2, S_CHUNK], R, tag="exp_row")
                    nc.scalar.activation(
                        exp_row[:],
                        attn_ps[:],
                        AF.Exp,
                        accum_out=zparts[0:2, c : c + 1],
                    )
                    # transpose exp to columns; accumulate context
                    for t in range(SUBS):
                        pt = tp_pool.tile([128, 2], R, tag="tp")
                        nc.tensor.matmul(
                            pt[:],
                            exp_row[0:2, ts(t, 128)],
                            ident[0:2, 0:2],
                            is_transpose=True,
                            start=True,
                            stop=True,
                        )
                        ec = small_pool.tile([128, 2], R, tag="ec")
                        nc.vector.tensor_copy(ec[:], pt[:])
                        for n in range(2):
                            nc.tensor.matmul(
                                ctx_ps[:, ts(n, 512)],
                                ec[:, 0:1],
                                nats[t][:, ts(n, 512)],
                                start=(c == 0 and t == 0),
                                stop=(c == n_chunk - 1 and t == SUBS - 1),
                            )
                # finalize batch: context / Z
                zsum = small_pool.tile([1, 1], F, tag="zsum")
                nc.vector.tensor_reduce(
                    zsum[:], zparts[0:1, :], mybir.AxisListType.X, mybir.AluOpType.add
                )
                rz = small_pool.tile([1, 1], F, tag="rz")
                nc.vector.reciprocal(rz[:], zsum[:])
                ctx_sb = small_pool.tile([1, ENC], F, tag="ctx_sb")
                nc.vector.tensor_scalar_mul(ctx_sb[:], ctx_ps[:], rz[:])
                nc.sync.dma_start(out[b : b + 1, :], ctx_sb[:])

    nc.compile()
    return nc


def kernel_run(hidden, encoder_outputs, attn_w, attn_b, v_w, **spmd_kwargs):
    """Shards over batch across 8 cores, runs the Bass kernel SPMD, gathers
    per-core outputs. Returns (full_output, BassKernelResults)."""
    global _CACHED_NC
    from concourse.bass_utils import run_bass_kernel_spmd

    if _CACHED_NC is None:
        _CACHED_NC = build_bass()
    nc = _CACHED_NC

    hidden = np.asarray(hidden, dtype=np.float32).reshape(BATCH, HIDDEN)
    enc = np.ascontiguousarray(np.asarray(encoder_outputs, dtype=np.float32))
    attn_w = np.ascontiguousarray(np.asarray(attn_w, dtype=np.float32))
    attn_b = np.ascontiguousarray(np.asarray(attn_b, dtype=np.float32))
    v_w = np.ascontiguousarray(np.asarray(v_w, dtype=np.float32))

    in_maps = []
    for c in range(NCORES):
        lo, hi = c * B_LOC, (c + 1) * B_LOC
        in_maps.append(
            {
                "hidden": np.ascontiguousarray(hidden[lo:hi]),
                "enc": np.ascontiguousarray(enc[lo:hi]),
                "attn_w": attn_w,
                "attn_b": attn_b,
                "v_w": v_w,
            }
        )

    res = run_bass_kernel_spmd(
        nc, in_maps, core_ids=list(range(NCORES)), **spmd_kwargs
    )
    outs = [r["out"] for r in res.results]
    full = np.concatenate(outs, axis=0).reshape(BATCH, 1, ENC)
    return full, res


def kernel(hidden, encoder_outputs, attn_w, attn_b, v_w):
    """Full-input entry point: takes the full (unsharded) inputs, returns the
    full [32, 1, 1024] output."""
    full, _ = kernel_run(hidden, encoder_outputs, attn_w, attn_b, v_w)
    return full
